# revision 1
# baseline (speedup 1.0000x reference)
"""3-layer GraphSAGE (mean aggr) on Trainium2, 8-core SPMD, fp16 compute.

Strategy (graph/data parallel per the sharding hint):
  - Nodes padded 10000 -> 10240 and assigned to 80 blocks of 128 by a
    degree-balancing permutation (host-side index work), so every block
    has ~2000 in-edges and needs exactly 16 edge chunks. Core r owns
    blocks [10r, 10r+10); one SPMD program runs on all 8 cores.
  - Per layer, source rows are fetched with GPSIMD dma_gather (fp16,
    256-512B rows; multi-packet mode -- single_packet crashes >8 chunks).
    Segment-mean runs on the PE as one-hot matmuls: gathered chunk rows
    as lhsT, a one-hot built on-device (iota==dst_local, 1/deg folded in,
    cached in SBUF and reused across layers) as rhs, accumulating mean^T
    in f32 PSUM. mean^T then feeds the layer matmul directly as lhsT.
  - Layer 2 emits h2 TRANSPOSED (w2l k-slices as lhsT), so h2 never
    touches DRAM; t3 = h2@[w3l|w3r] (layer-3 "transform first") runs in
    the same block loop and only t3 [10240,128pad] fp16 is AllGathered,
    never h2 [10240,1024]. The root half w2r.h1^T is precomputed into
    SBUF while the h1 AllGather is in flight (it only needs local data)
    and added back on DVE, shortening the post-AllGather critical path.
  - h1^T, r3, and the one-hot cache stay SBUF-resident across phases.
    Cross-core traffic is two AllGathers: h1 (5.2MB fp16), t3 (2.6MB).
  - log_softmax per node on DVE/ACT in f32; per-core output slices are
    concatenated and un-permuted on host. PSUM accumulation is f32;
    end-to-end vs the f32 reference: max abs err ~2e-3 (rel ~2.6e-4).
"""

import numpy as np
BF = np.float16

N_NODES = 10000
NPAD = 10240
NCORES = 8
P = 128
NB = 10                      # dst blocks per core
PER_CORE = NB * P            # 1280
D_IN, D_H1, D_H2, D_OUT = 128, 256, 1024, 64

_CACHE = {}
LAST_RESULTS = None          # test harness reads exec_time_ns from here


def _build(MC):
    import os
    import concourse.bacc as bacc
    import concourse.mybir as mybir
    import concourse.tile as tile

    abl = set(os.environ.get("KABL", "").split(","))

    f32 = mybir.dt.float32
    bf16 = mybir.dt.float16
    i16 = mybir.dt.int16
    nc = bacc.Bacc("TRN2", target_bir_lowering=False, debug=False,
                   num_devices=NCORES)

    xbf = nc.dram_tensor("xbf", [NPAD, D_IN], bf16, kind="ExternalInput")
    xownT = nc.dram_tensor("xownT", [P, PER_CORE], bf16, kind="ExternalInput")
    w1l = nc.dram_tensor("w1l", [D_IN, D_H1], bf16, kind="ExternalInput")
    w1r = nc.dram_tensor("w1r", [D_IN, D_H1], bf16, kind="ExternalInput")
    b1 = nc.dram_tensor("b1", [1, D_H1], bf16, kind="ExternalInput")
    b1t = nc.dram_tensor("b1t", [P, 2], f32, kind="ExternalInput")
    w2l = nc.dram_tensor("w2l", [D_H1, D_H2], bf16, kind="ExternalInput")
    w2r = nc.dram_tensor("w2r", [D_H1, D_H2], bf16, kind="ExternalInput")
    b2t = nc.dram_tensor("b2t", [P, 8], f32, kind="ExternalInput")
    w3lr = nc.dram_tensor("w3lr", [D_H2, P], bf16, kind="ExternalInput")
    b3pad = nc.dram_tensor("b3pad", [1, P], bf16, kind="ExternalInput")
    gidx = nc.dram_tensor("gidx", [P, NB * MC * 8], i16, kind="ExternalInput")
    dstloc = nc.dram_tensor("dstloc", [P, NB * MC], f32, kind="ExternalInput")
    invdeg = nc.dram_tensor("invdeg", [P, NB * MC], f32, kind="ExternalInput")
    iota_in = nc.dram_tensor("iota_in", [P, P], f32, kind="ExternalInput")
    outp = nc.dram_tensor("out", [PER_CORE, D_OUT], f32, kind="ExternalOutput")

    EXP = mybir.ActivationFunctionType.Exp
    LN = mybir.ActivationFunctionType.Ln
    RELU = mybir.ActivationFunctionType.Relu
    EQ = mybir.AluOpType.is_equal
    MUL = mybir.AluOpType.mult
    SUB = mybir.AluOpType.subtract
    ADD = mybir.AluOpType.add
    MAX = mybir.AluOpType.max
    AXX = mybir.AxisListType.X

    with tile.TileContext(nc) as tc:
        with (
            tc.tile_pool(name="const", bufs=1) as cp,
            tc.tile_pool(name="gath", bufs=3) as gp,
            tc.tile_pool(name="ht", bufs=3) as htp,
            tc.tile_pool(name="meant", bufs=3) as mtp,
            tc.tile_pool(name="hout", bufs=2) as hop,
            tc.tile_pool(name="small", bufs=6) as smp,
            tc.tile_pool(name="psA", bufs=2, space="PSUM") as psA,
            tc.tile_pool(name="psO", bufs=2, space="PSUM") as psO,
            tc.tile_pool(name="psT", bufs=4, space="PSUM") as psT,
            tc.tile_pool(name="dram", bufs=1, space="DRAM") as dram,
        ):
            # ---- constants ----
            iota_t = cp.tile([P, P], f32, tag="iota")
            nc.sync.dma_start(iota_t[:], iota_in[:])
            ones_t = cp.tile([1, P], bf16, tag="ones")
            nc.vector.memset(ones_t[:], 1.0)
            gidx_sb = cp.tile([P, NB * MC * 8], i16, tag="gidx")
            nc.sync.dma_start(gidx_sb[:], gidx[:])
            dstloc_sb = cp.tile([P, NB * MC], f32, tag="dstloc")
            nc.sync.dma_start(dstloc_sb[:], dstloc[:])
            invdeg_sb = cp.tile([P, NB * MC], f32, tag="invdeg")
            nc.sync.dma_start(invdeg_sb[:], invdeg[:])

            # weights arrive fp16 from the host; plain HWDGE loads
            w1l_sb = cp.tile([P, D_H1], bf16, tag="w1l")
            nc.sync.dma_start(w1l_sb[:], w1l[:])
            w1r_sb = cp.tile([P, D_H1], bf16, tag="w1r")
            nc.sync.dma_start(w1r_sb[:], w1r[:])
            b1_sb = cp.tile([1, D_H1], bf16, tag="b1")
            nc.sync.dma_start(b1_sb[:], b1[:])
            b1t_sb = cp.tile([P, 2], f32, tag="b1t")
            nc.sync.dma_start(b1t_sb[:], b1t[:])

            # resident cross-phase SBUF state
            xT_res = cp.tile([P, PER_CORE], bf16, tag="xT")
            nc.sync.dma_start(xT_res[:], xownT[:])
            h1T_res = cp.tile([P, 2, PER_CORE], bf16, tag="h1T")
            r3_res = cp.tile([P, NB, D_OUT], f32, tag="r3")
            ohc = cp.tile([P, NB * MC, P], bf16, tag="ohc")

            # ---- DRAM intermediates ----
            h1_own = dram.tile([PER_CORE, D_H1], bf16, tag="h1o")
            h1_full = dram.tile([NPAD, D_H1], bf16, tag="h1f")
            t3_own = dram.tile([PER_CORE, P], bf16, tag="t3o")
            t3_full = dram.tile([NPAD, P], bf16, tag="t3f")

            # single_packet=True caps one gather at 64 descs/engine (8
            # chunks) and crashes beyond; multi-packet handles 18+ chunks.
            # 8-chunk sub-gathers measured fastest (finer DMA/compute
            # interleave at phase starts vs per-gather fixed overhead).
            def gather_block(dst_tile, src_ap, b, elem, GSZ=8):
                if "nogather" in abl:
                    return
                g0 = 0
                while g0 < MC:
                    gsz = min(GSZ, MC - g0)
                    c0 = (b * MC + g0) * 8
                    nc.gpsimd.dma_gather(
                        dst_tile[:, g0:g0 + gsz, :], src_ap,
                        gidx_sb[:, c0:c0 + gsz * 8],
                        gsz * P, gsz * P, elem, single_packet=False)
                    g0 += gsz

            # ================= Layer 1 =================
            for b in range(NB if "l1" not in abl else 0):
                gath = gp.tile([P, MC, D_IN], bf16, tag="gath")
                gather_block(gath, xbf[:], b, D_IN)
                # build + cache this block's one-hots (bf16, reused in L2)
                for c in range(MC if "nooh" not in abl else 0):
                    col = b * MC + c
                    nc.vector.tensor_scalar(
                        ohc[:, col, :], iota_t[:],
                        dstloc_sb[:, col:col + 1], invdeg_sb[:, col:col + 1],
                        EQ, MUL)
                agg = psA.tile([P, 2 * P], f32, tag="agg")
                for c in range(MC if "noagg" not in abl else 0):
                    nc.tensor.matmul(agg[:, 0:P], gath[:, c, :],
                                     ohc[:, b * MC + c, :],
                                     start=(c == 0), stop=(c == MC - 1))
                meanT = mtp.tile([P, 2, P], bf16, tag="meanT")
                nc.vector.tensor_copy(meanT[:, 0, :], agg[:, 0:P])

                xT = xT_res[:, b * P:(b + 1) * P]

                # node-major h1 (for AllGather/gather)
                op = psO.tile([P, 2 * D_H1], f32, tag="outp")
                nc.tensor.matmul(op[:, 0:D_H1], meanT[:, 0, :], w1l_sb[:],
                                 start=True, stop=False)
                nc.tensor.matmul(op[:, 0:D_H1], xT, w1r_sb[:],
                                 start=False, stop=False)
                nc.tensor.matmul(op[:, 0:D_H1], ones_t[:], b1_sb[:],
                                 start=False, stop=True)
                h1blk = hop.tile([P, D_H1], bf16, tag="hout")
                nc.scalar.activation(h1blk[:], op[:, 0:D_H1], RELU)
                nc.sync.dma_start(h1_own[b * P:(b + 1) * P, :], h1blk[:])
                # transposed h1 (for the L2 root term), SBUF-resident
                for s in range(2):
                    tph = psT.tile([P, P], f32, tag="tp")
                    nc.tensor.matmul(tph[:], w1l_sb[:, s * P:(s + 1) * P],
                                     meanT[:, 0, :], start=True, stop=False)
                    nc.tensor.matmul(tph[:], w1r_sb[:, s * P:(s + 1) * P],
                                     xT, start=False, stop=True)
                    nc.scalar.activation(h1T_res[:, s, b * P:(b + 1) * P],
                                         tph[:], RELU,
                                         bias=b1t_sb[:, s:s + 1])

            # layer-2/3 weights are first needed after L1: emit their loads
            # here so they don't delay the L1 gathers in the SWDGE queue
            w2l_sb = cp.tile([P, 2, D_H2], bf16, tag="w2l")
            nc.sync.dma_start(w2l_sb[:], w2l.rearrange("(s p) n -> p s n", p=P))
            w2r_sb = cp.tile([P, 2, D_H2], bf16, tag="w2r")
            nc.sync.dma_start(w2r_sb[:], w2r.rearrange("(s p) n -> p s n", p=P))
            b2t_sb = cp.tile([P, 8], f32, tag="b2t")
            nc.sync.dma_start(b2t_sb[:], b2t[:])
            w3lr_sb = cp.tile([P, 8, P], bf16, tag="w3lr")
            nc.sync.dma_start(w3lr_sb[:], w3lr.rearrange("(s p) n -> p s n", p=P))
            b3_sb = cp.tile([1, P], bf16, tag="b3")
            nc.sync.dma_start(b3_sb[:], b3pad[:])

            # root half of h2^T depends only on local h1T_res: compute it
            # while the h1 AllGather is in flight, staged to SBUF.
            h2root = cp.tile([P, NB, 8, P], f32, tag="h2r")
            for b in range(NB if "l2" not in abl else 0):
                for s in range(8):
                    hp = psT.tile([P, P], f32, tag="tp")
                    for k in range(2):
                        nc.tensor.matmul(
                            hp[:], w2r_sb[:, k, s * P:(s + 1) * P],
                            h1T_res[:, k, b * P:(b + 1) * P],
                            start=(k == 0), stop=(k == 1))
                    nc.vector.tensor_copy(h2root[:, b, s, :], hp[:])

            if "noag" not in abl:
                nc.gpsimd.collective_compute(
                    "AllGather", mybir.AluOpType.bypass,
                    replica_groups=[list(range(NCORES))],
                    ins=[h1_own.opt()], outs=[h1_full.opt()])

            # ====== Layer 2 fused with L3 transform (h2 never in DRAM) =====
            for b in range(NB if "l2" not in abl else 0):
                gath = gp.tile([P, MC, D_H1], bf16, tag="gath")
                gather_block(gath, h1_full[:], b, D_H1, GSZ=6)
                agg = psA.tile([P, 2 * P], f32, tag="agg")
                for s in range(2 if "noagg" not in abl else 0):
                    for c in range(MC):
                        nc.tensor.matmul(agg[:, s * P:(s + 1) * P],
                                         gath[:, c, s * P:(s + 1) * P],
                                         ohc[:, b * MC + c, :],
                                         start=(c == 0), stop=(c == MC - 1))
                meanT = mtp.tile([P, 2, P], bf16, tag="meanT")
                nc.vector.tensor_copy(meanT[:, 0, :], agg[:, 0:P])
                nc.vector.tensor_copy(meanT[:, 1, :], agg[:, P:2 * P])

                # h2^T slices: mean half on PE, + staged root half on DVE
                hT = htp.tile([P, 8, P], bf16, tag="hT")
                for s in range(8 if "noout" not in abl else 0):
                    hp = psT.tile([P, P], f32, tag="tp")
                    for k in range(2):
                        nc.tensor.matmul(
                            hp[:], w2l_sb[:, k, s * P:(s + 1) * P],
                            meanT[:, k, :], start=(k == 0), stop=(k == 1))
                    ysum = smp.tile([P, P], f32, tag="ysum")
                    nc.vector.tensor_tensor(ysum[:], hp[:],
                                            h2root[:, b, s, :], ADD)
                    nc.scalar.activation(hT[:, s, :], ysum[:], RELU,
                                         bias=b2t_sb[:, s:s + 1])

                # [t3 | r3] = h2 @ [w3l | w3r] + [0 | b3] (lhsT = h2^T)
                tr = psO.tile([P, 2 * D_H1], f32, tag="outp")
                for s in range(8):
                    nc.tensor.matmul(tr[:, 0:P], hT[:, s, :],
                                     w3lr_sb[:, s, :],
                                     start=(s == 0), stop=False)
                nc.tensor.matmul(tr[:, 0:P], ones_t[:], b3_sb[:],
                                 start=False, stop=True)
                t3blk = smp.tile([P, P], bf16, tag="t3blk")
                nc.vector.tensor_copy(t3blk[:, 0:D_OUT], tr[:, 0:D_OUT])
                nc.vector.memset(t3blk[:, D_OUT:P], 0.0)
                nc.vector.tensor_copy(r3_res[:, b, :], tr[:, D_OUT:P])
                nc.sync.dma_start(t3_own[b * P:(b + 1) * P, :], t3blk[:])

            if "noag" not in abl:
                nc.gpsimd.collective_compute(
                    "AllGather", mybir.AluOpType.bypass,
                    replica_groups=[list(range(NCORES))],
                    ins=[t3_own.opt()], outs=[t3_full.opt()])

            # ================= Layer 3 aggregate + log_softmax ==============
            for b in range(NB if "l3b" not in abl else 0):
                gath = gp.tile([P, MC, P], bf16, tag="gath")
                gather_block(gath, t3_full[:], b, P)
                agg = psA.tile([P, 2 * P], f32, tag="agg")
                for c in range(MC if "noagg" not in abl else 0):
                    nc.tensor.matmul(agg[:, 0:D_OUT], ohc[:, b * MC + c, :],
                                     gath[:, c, 0:D_OUT],
                                     start=(c == 0), stop=(c == MC - 1))
                y = smp.tile([P, D_OUT], f32, tag="y")
                nc.vector.tensor_tensor(y[:], agg[:, 0:D_OUT],
                                        r3_res[:, b, :], ADD)
                negm = smp.tile([P, 1], f32, tag="negm")
                nc.vector.tensor_reduce(negm[:], y[:], AXX, MAX, negate=True)
                e = smp.tile([P, D_OUT], f32, tag="e")
                ssum = smp.tile([P, 1], f32, tag="ssum")
                nc.scalar.activation(e[:], y[:], EXP, bias=negm[:, 0:1],
                                     scale=1.0, accum_out=ssum[:])
                ls = smp.tile([P, 1], f32, tag="ls")
                nc.scalar.activation(ls[:], ssum[:], LN)
                ob = smp.tile([P, D_OUT], f32, tag="ob")
                nc.vector.tensor_scalar(ob[:], y[:], negm[:, 0:1],
                                        ls[:, 0:1], ADD, SUB)
                nc.sync.dma_start(outp[b * P:(b + 1) * P, :], ob[:])

    nc.compile()
    return nc


def _wrap16(a):
    """idx i -> partition i%16, col i//16; replicated to 128 partitions."""
    w = a.reshape(-1, 16).T
    return np.ascontiguousarray(np.tile(w, (8, 1)))


def _balanced_perm(deg):
    """Assign nodes to 80 blocks of 128 so block in-degree sums are even.

    Greedy: highest-degree node goes to the open block with the lowest
    degree sum. Returns newpos[old_node] -> permuted node id. This makes
    every block need exactly ceil(E/(NBLK*P)) = 16 edge chunks instead of
    the unbalanced max (17+), trimming gather + aggregation work ~6%.
    """
    import heapq
    nblk = NPAD // P
    order = np.argsort(-deg, kind="stable")
    heap = [(0, 0, g) for g in range(nblk)]
    heapq.heapify(heap)
    newpos = np.empty(NPAD, np.int64)
    fill = np.zeros(nblk, np.int64)
    for n in order:
        s, _, g = heapq.heappop(heap)
        newpos[n] = g * P + fill[g]
        fill[g] += 1
        if fill[g] < P:
            heapq.heappush(heap, (s + int(deg[n]), int(fill[g]), g))
    return newpos


def _prep(x, edge_index):
    src = np.asarray(edge_index[0], dtype=np.int64)
    dst = np.asarray(edge_index[1], dtype=np.int64)
    deg = np.bincount(dst, minlength=NPAD).astype(np.float64)
    invdeg_n = (1.0 / np.maximum(deg, 1.0)).astype(np.float32)

    newpos = _balanced_perm(deg)
    oldnode = np.empty(NPAD, np.int64)
    oldnode[newpos] = np.arange(NPAD)
    psrc = newpos[src]
    pdst = newpos[dst]

    order = np.argsort(pdst, kind="stable")
    dsts = pdst[order]
    srcs = psrc[order]
    inv_e = invdeg_n[dst[order]]
    starts = np.searchsorted(dsts, np.arange(0, NPAD + P, P))
    cnt = starts[1:] - starts[:-1]
    MC = max(1, int(np.ceil(cnt.max() / P)))

    xp = np.zeros((NPAD, D_IN), dtype=np.float32)
    xp[:N_NODES] = x
    xp = xp[oldnode]           # permuted node order

    per_core = []
    for r in range(NCORES):
        gparts, dparts, iparts = [], [], []
        for j in range(NB):
            g = r * NB + j
            lo, hi = starts[g], starts[g + 1]
            n = hi - lo
            # ascending source addresses -> better HBM locality in the
            # gather's descriptor stream (aggregation is order-invariant)
            o2 = lo + np.argsort(srcs[lo:hi], kind="stable")
            sg = np.zeros(MC * P, dtype=np.int16)
            dg = np.full(MC * P, -1.0, dtype=np.float32)
            ig = np.zeros(MC * P, dtype=np.float32)
            sg[:n] = srcs[o2].astype(np.int16)
            dg[:n] = (dsts[o2] - g * P).astype(np.float32)
            ig[:n] = inv_e[o2]
            gparts.append(_wrap16(sg))
            dparts.append(np.ascontiguousarray(dg.reshape(MC, P).T))
            iparts.append(np.ascontiguousarray(ig.reshape(MC, P).T))
        per_core.append((
            np.concatenate(gparts, axis=1),
            np.concatenate(dparts, axis=1),
            np.concatenate(iparts, axis=1),
        ))
    return xp, per_core, MC, newpos


def _make_in_maps(x, edge_index, w1l, w1r, b1, w2l, w2r, b2, w3l, w3r, b3):
    x = np.ascontiguousarray(np.asarray(x, dtype=np.float32))
    xp, per_core, MC, newpos = _prep(x, np.asarray(edge_index))

    iota = np.tile(np.arange(P, dtype=np.float32), (P, 1))
    b1v = np.asarray(b1, np.float32).reshape(-1)
    b2v = np.asarray(b2, np.float32).reshape(-1)
    xbf = xp.astype(BF)
    common = {
        "xbf": xbf,
        "w1l": np.asarray(w1l, np.float32).astype(BF),
        "w1r": np.asarray(w1r, np.float32).astype(BF),
        "b1": b1v.reshape(1, D_H1).astype(BF),
        "b1t": np.ascontiguousarray(b1v.reshape(2, P).T),
        "w2l": np.asarray(w2l, np.float32).astype(BF),
        "w2r": np.asarray(w2r, np.float32).astype(BF),
        "b2t": np.ascontiguousarray(b2v.reshape(8, P).T),
        "w3lr": np.ascontiguousarray(np.concatenate(
            [np.asarray(w3l, np.float32), np.asarray(w3r, np.float32)],
            axis=1)).astype(BF),
        "b3pad": np.concatenate(
            [np.zeros(D_OUT, np.float32),
             np.asarray(b3, np.float32).reshape(-1)]).reshape(1, P).astype(BF),
        "iota_in": iota,
    }
    in_maps = []
    for r in range(NCORES):
        g, d, iv = per_core[r]
        m = dict(common)
        m["xownT"] = np.ascontiguousarray(
            xbf[r * PER_CORE:(r + 1) * PER_CORE].T)
        m["gidx"] = g
        m["dstloc"] = d
        m["invdeg"] = iv
        in_maps.append(m)
    return in_maps, MC, newpos


def kernel(x, edge_index, w1l, w1r, b1, w2l, w2r, b2, w3l, w3r, b3):
    global LAST_RESULTS
    import os
    from concourse.bass_utils import run_bass_kernel_spmd

    if os.environ.get("BASS_TRACE"):
        try:
            import antenv.axon_hooks  # noqa: F401
        except ImportError:
            os.environ.pop("BASS_TRACE", None)  # no NTFF hook here

    in_maps, MC, newpos = _make_in_maps(x, edge_index, w1l, w1r, b1, w2l,
                                        w2r, b2, w3l, w3r, b3)
    if MC not in _CACHE:
        _CACHE[MC] = _build(MC)
    nc = _CACHE[MC]

    res = run_bass_kernel_spmd(nc, in_maps, core_ids=list(range(NCORES)))
    LAST_RESULTS = res
    out = np.concatenate([res.results[r]["out"] for r in range(NCORES)], axis=0)
    return np.ascontiguousarray(out[newpos[:N_NODES]])



# revision 8
# speedup vs baseline: 1.3403x; 1.3403x over previous
"""3-layer GraphSAGE (mean aggr) on Trainium2, 8-core SPMD, fp16 compute.

Strategy (graph/data parallel, ReduceScatter formulation):
  - Nodes padded 10000 -> 10240, degree-balancing permutation assigns them
    to 80 blocks of 128 (block in-degree ~2000 -> MC1=16 in-edge chunks).
    Core r owns blocks [10r, 10r+10); one SPMD program on all 8 cores.
  - L1 (x replicated): per own dst block, gather x[src] rows (GPSIMD
    dma_gather), aggregate via one-hot matmuls into node-major sums,
    scale by 1/deg (DVE), transpose on PE (identity matmul) -> mean1^T,
    then h1 = relu(mean1@w1l + x@w1r + b1) both node-major (DRAM, gather
    source for L2) and transposed (SBUF-resident, L2 root term).
  - L2/L3 aggregation is SOURCE-partitioned: each core computes partial
    neighbor SUMS for ALL 80 dst blocks from its OWN h1/t3 rows (local
    gathers + pure-0/1 one-hot matmuls, PSUM f32), written block-major to
    DRAM; one ReduceScatter(add) then delivers each core its own rows.
    RS moves 655KB (L2) + 160KB (L3) instead of AllGather's 5.2MB+2.6MB:
    collective time drops 227us -> ~51us. 1/deg is applied after the RS
    (per-partition scale), so one-hots stay exact {0,1} and are built 16
    chunks at a time with one broadcast DVE op.
  - L2 own-block: mean2^T via PE transpose, h2^T slices from 4 accumulated
    matmuls (w2l x mean + w2r x h1^T), relu+bias on DVE; t3 = h2@[w3l|w3r]
    in the same loop; t3 node-major to DRAM for the L3 partial pass.
  - L3: same local chunk structure as L2 (one-hot cache + gather indices
    reused), partial sums [80,128,64] -> RS -> own rows; out = sums/deg +
    r3, log_softmax with a single deferred Ln over all blocks' accumulated
    exp-sums (2 ACT table loads total instead of 20).
  - All relu/bias work runs on DVE (tensor_scalar ADD,MAX), keeping the
    ACT engine out of the critical path entirely except Exp/Ln.
"""

import numpy as np
BF = np.float16

N_NODES = 10000
NPAD = 10240
NCORES = 8
P = 128
NB = 10                      # own dst blocks per core
NBLK = NPAD // P             # 80 global blocks
PER_CORE = NB * P            # 1280
D_IN, D_H1, D_H2, D_OUT = 128, 256, 1024, 64
GSZ2 = 16                    # chunks per L2/L3 partial gather group
AHEAD = 16                   # chunk lookahead for gather emission

_CACHE = {}
LAST_RESULTS = None          # test harness reads exec_time_ns from here


def _build(MC1, C2):
    import os
    import concourse.bacc as bacc
    import concourse.mybir as mybir
    import concourse.tile as tile

    abl = set(os.environ.get("KABL", "").split(","))

    f32 = mybir.dt.float32
    bf16 = mybir.dt.float16
    i16 = mybir.dt.int16
    nc = bacc.Bacc("TRN2", target_bir_lowering=False, debug=False,
                   num_devices=NCORES)

    NCH2 = int(sum(C2))
    cstart = [0]
    for c in C2:
        cstart.append(cstart[-1] + int(c))
    # gather groups over the L2/L3 chunk stream
    groups = []
    lo = 0
    while lo < NCH2:
        n = min(GSZ2, NCH2 - lo)
        groups.append((lo, n))
        lo += n
    chunk_grp = np.zeros(NCH2, np.int64)
    chunk_off = np.zeros(NCH2, np.int64)
    for gi, (glo, gn) in enumerate(groups):
        chunk_grp[glo:glo + gn] = gi
        chunk_off[glo:glo + gn] = np.arange(gn)

    xbf = nc.dram_tensor("xbf", [NPAD, D_IN], bf16, kind="ExternalInput")
    xownT = nc.dram_tensor("xownT", [P, PER_CORE], bf16, kind="ExternalInput")
    w1l = nc.dram_tensor("w1l", [D_IN, D_H1], bf16, kind="ExternalInput")
    w1r = nc.dram_tensor("w1r", [D_IN, D_H1], bf16, kind="ExternalInput")
    b1 = nc.dram_tensor("b1", [1, D_H1], bf16, kind="ExternalInput")
    b1t = nc.dram_tensor("b1t", [P, 2], f32, kind="ExternalInput")
    w2l = nc.dram_tensor("w2l", [D_H1, D_H2], bf16, kind="ExternalInput")
    w2r = nc.dram_tensor("w2r", [D_H1, D_H2], bf16, kind="ExternalInput")
    b2t = nc.dram_tensor("b2t", [P, 8], f32, kind="ExternalInput")
    w3lr = nc.dram_tensor("w3lr", [D_H2, P], bf16, kind="ExternalInput")
    b3pad = nc.dram_tensor("b3pad", [1, P], bf16, kind="ExternalInput")
    gidx1 = nc.dram_tensor("gidx1", [P, NB * MC1 * 8], i16, kind="ExternalInput")
    dstloc1 = nc.dram_tensor("dstloc1", [P, NB * MC1], f32, kind="ExternalInput")
    gidx2 = nc.dram_tensor("gidx2", [P, NCH2 * 8], i16, kind="ExternalInput")
    dstloc2 = nc.dram_tensor("dstloc2", [P, NCH2], f32, kind="ExternalInput")
    invd_own = nc.dram_tensor("invd_own", [P, NB], f32, kind="ExternalInput")
    iota_in = nc.dram_tensor("iota_in", [P, P], f32, kind="ExternalInput")
    partidx = nc.dram_tensor("partidx", [P, 1], f32, kind="ExternalInput")
    outp = nc.dram_tensor("out", [PER_CORE, D_OUT], f32, kind="ExternalOutput")

    EXP = mybir.ActivationFunctionType.Exp
    LN = mybir.ActivationFunctionType.Ln
    EQ = mybir.AluOpType.is_equal
    MUL = mybir.AluOpType.mult
    SUB = mybir.AluOpType.subtract
    ADD = mybir.AluOpType.add
    MAX = mybir.AluOpType.max
    AXX = mybir.AxisListType.X

    with tile.TileContext(nc) as tc:
        with (
            tc.tile_pool(name="const", bufs=1) as cp,
            tc.tile_pool(name="g1", bufs=3) as gp1,
            tc.tile_pool(name="g2", bufs=3) as gp2,
            tc.tile_pool(name="g3", bufs=3) as gp3,
            tc.tile_pool(name="oh1", bufs=2) as ohp,
            tc.tile_pool(name="small", bufs=6) as smp,
            tc.tile_pool(name="stage", bufs=2) as stp,
            tc.tile_pool(name="hts", bufs=2) as htp,
            tc.tile_pool(name="psA", bufs=2, space="PSUM") as psA,
            tc.tile_pool(name="psO", bufs=2, space="PSUM") as psO,
            tc.tile_pool(name="psT", bufs=2, space="PSUM") as psT,
            tc.tile_pool(name="dram", bufs=1, space="DRAM") as dram,
        ):
            # ---- constants ----
            iota_t = cp.tile([P, P], f32, tag="iota")
            nc.sync.dma_start(iota_t[:], iota_in[:])
            pidx_t = cp.tile([P, 1], f32, tag="pidx")
            nc.sync.dma_start(pidx_t[:], partidx[:])
            ones_t = cp.tile([1, P], bf16, tag="ones")
            nc.vector.memset(ones_t[:], 1.0)
            ident = cp.tile([P, P], bf16, tag="ident")
            nc.vector.tensor_scalar(ident[:], iota_t[:], pidx_t[:, 0:1], None, EQ)
            gidx1_sb = cp.tile([P, NB * MC1 * 8], i16, tag="gidx1")
            nc.sync.dma_start(gidx1_sb[:], gidx1[:])
            gidx2_sb = cp.tile([P, NCH2 * 8], i16, tag="gidx2")
            nc.sync.dma_start(gidx2_sb[:], gidx2[:])
            dstloc1_sb = cp.tile([P, NB * MC1], f32, tag="dstloc1")
            nc.sync.dma_start(dstloc1_sb[:], dstloc1[:])
            dstloc2_sb = cp.tile([P, NCH2], f32, tag="dstloc2")
            nc.sync.dma_start(dstloc2_sb[:], dstloc2[:])
            invd_sb = cp.tile([P, NB], f32, tag="invd")
            nc.sync.dma_start(invd_sb[:], invd_own[:])

            w1l_sb = cp.tile([P, D_H1], bf16, tag="w1l")
            nc.sync.dma_start(w1l_sb[:], w1l[:])
            w1r_sb = cp.tile([P, D_H1], bf16, tag="w1r")
            nc.sync.dma_start(w1r_sb[:], w1r[:])
            b1_sb = cp.tile([1, D_H1], bf16, tag="b1")
            nc.sync.dma_start(b1_sb[:], b1[:])
            b1t_sb = cp.tile([P, 2], f32, tag="b1t")
            nc.sync.dma_start(b1t_sb[:], b1t[:])

            # resident cross-phase SBUF state
            xT_res = cp.tile([P, PER_CORE], bf16, tag="xT")
            nc.sync.dma_start(xT_res[:], xownT[:])
            h1T_res = cp.tile([P, 2, PER_CORE], bf16, tag="h1T")
            r3_res = cp.tile([P, NB, D_OUT], f32, tag="r3")
            ob_res = cp.tile([P, NB, D_OUT], f32, tag="ob")
            ssum_res = cp.tile([P, NB], f32, tag="ssum")
            ls_res = cp.tile([P, NB], f32, tag="ls")
            ohc2 = cp.tile([P, NCH2, P], bf16, tag="ohc2")

            # ---- DRAM intermediates ----
            h1_own = dram.tile([PER_CORE, D_H1], bf16, tag="h1o")
            t3_own = dram.tile([PER_CORE, P], bf16, tag="t3o")
            p2 = dram.tile([NBLK, P, D_H1], bf16, tag="p2")
            m2 = dram.tile([NB, P, D_H1], bf16, tag="m2")
            p3 = dram.tile([NBLK, P, D_OUT], bf16, tag="p3")
            m3 = dram.tile([NB, P, D_OUT], bf16, tag="m3")

            # ================= Layer 1 (own dst blocks) =================
            GSZ1 = 8
            for b in range(NB if "l1" not in abl else 0):
                gath = gp1.tile([P, MC1, D_IN], bf16, tag="gath1")
                g0 = 0
                while g0 < MC1:
                    gsz = min(GSZ1, MC1 - g0)
                    c0 = (b * MC1 + g0) * 8
                    nc.gpsimd.dma_gather(
                        gath[:, g0:g0 + gsz, :], xbf[:],
                        gidx1_sb[:, c0:c0 + gsz * 8],
                        gsz * P, gsz * P, D_IN, single_packet=False)
                    g0 += gsz
                # one-hot (0/1) for the block's chunks: one broadcast DVE op
                oh1 = ohp.tile([P, MC1, P], bf16, tag="oh1")
                c0 = b * MC1
                nc.vector.tensor_tensor(
                    oh1[:],
                    dstloc1_sb[:, c0:c0 + MC1].unsqueeze(2)
                        .broadcast_to([P, MC1, P]),
                    iota_t[:].unsqueeze(1).broadcast_to([P, MC1, P]),
                    EQ)
                # node-major neighbor sums for this block
                agg = psA.tile([P, D_IN], f32, tag="agg1")
                for c in range(MC1):
                    nc.tensor.matmul(agg[:], oh1[:, c, :],
                                     gath[:, c, :],
                                     start=(c == 0), stop=(c == MC1 - 1))
                # mean = sums/deg (DVE, per-partition scale), then PE transpose
                m1s = smp.tile([P, D_IN], bf16, tag="m1s")
                nc.vector.tensor_scalar(m1s[:], agg[:],
                                        invd_sb[:, b:b + 1], None, MUL)
                tp1 = psT.tile([P, P], f32, tag="tp")
                nc.tensor.matmul(tp1[:], m1s[:], ident[:], start=True, stop=True)
                meanT = smp.tile([P, D_IN], bf16, tag="meanT1")
                nc.vector.tensor_copy(meanT[:], tp1[:])

                xT = xT_res[:, b * P:(b + 1) * P]

                # node-major h1 (gather source for the L2 partial pass)
                op = psO.tile([P, D_H1], f32, tag="outp")
                nc.tensor.matmul(op[:], meanT[:], w1l_sb[:],
                                 start=True, stop=False)
                nc.tensor.matmul(op[:], xT, w1r_sb[:], start=False, stop=False)
                nc.tensor.matmul(op[:], ones_t[:], b1_sb[:],
                                 start=False, stop=True)
                h1blk = smp.tile([P, D_H1], bf16, tag="h1blk")
                nc.vector.tensor_scalar(h1blk[:], op[:], 0.0, None, MAX)
                nc.sync.dma_start(h1_own[b * P:(b + 1) * P, :], h1blk[:])
                # transposed h1 (L2 root term), SBUF-resident
                for s in range(2):
                    tph = psT.tile([P, P], f32, tag="tp")
                    nc.tensor.matmul(tph[:], w1l_sb[:, s * P:(s + 1) * P],
                                     meanT[:], start=True, stop=False)
                    nc.tensor.matmul(tph[:], w1r_sb[:, s * P:(s + 1) * P],
                                     xT, start=False, stop=True)
                    nc.vector.tensor_scalar(h1T_res[:, s, b * P:(b + 1) * P],
                                            tph[:], b1t_sb[:, s:s + 1], 0.0,
                                            ADD, MAX)

            # layer-2/3 weights first needed after L1
            w2l_sb = cp.tile([P, 2, D_H2], bf16, tag="w2l")
            nc.sync.dma_start(w2l_sb[:], w2l.rearrange("(s p) n -> p s n", p=P))
            w2r_sb = cp.tile([P, 2, D_H2], bf16, tag="w2r")
            nc.sync.dma_start(w2r_sb[:], w2r.rearrange("(s p) n -> p s n", p=P))
            b2t_sb = cp.tile([P, 8], f32, tag="b2t")
            nc.sync.dma_start(b2t_sb[:], b2t[:])
            w3lr_sb = cp.tile([P, 8, P], bf16, tag="w3lr")
            nc.sync.dma_start(w3lr_sb[:], w3lr.rearrange("(s p) n -> p s n", p=P))
            b3_sb = cp.tile([1, P], bf16, tag="b3")
            nc.sync.dma_start(b3_sb[:], b3pad[:])

            # ========== Layer 2 partial sums for ALL dst blocks ==========
            state = {"next": 0, "tiles": {}}

            def need_chunks(pool, src_ap, elem, tag, target, build):
                while (state["next"] < len(groups)
                       and groups[state["next"]][0] < target):
                    glo, gn = groups[state["next"]]
                    if build:
                        nc.vector.tensor_tensor(
                            ohc2[:, glo:glo + gn, :],
                            dstloc2_sb[:, glo:glo + gn].unsqueeze(2)
                                .broadcast_to([P, gn, P]),
                            iota_t[:].unsqueeze(1).broadcast_to([P, gn, P]),
                            EQ)
                    t = pool.tile([P, GSZ2, elem], bf16, tag=tag)
                    nc.gpsimd.dma_gather(
                        t[:, 0:gn, :], src_ap,
                        gidx2_sb[:, glo * 8:(glo + gn) * 8],
                        gn * P, gn * P, elem, single_packet=False)
                    state["tiles"][state["next"]] = t
                    state["next"] += 1

            if "l2p" not in abl:
                state["next"] = 0
                state["tiles"] = {}
                agg2 = None
                for g in range(NBLK):
                    c0, cn = cstart[g], int(C2[g])
                    need_chunks(gp2, h1_own[:], D_H1, "gath2",
                                min(c0 + cn + AHEAD, NCH2), True)
                    q = g % 2
                    if q == 0:
                        agg2 = psA.tile([P, 2, D_H1], f32, tag="agg2")
                    for ci in range(cn):
                        c = c0 + ci
                        gt = state["tiles"][int(chunk_grp[c])]
                        nc.tensor.matmul(
                            agg2[:, q, :], ohc2[:, c, :],
                            gt[:, int(chunk_off[c]), :],
                            start=(ci == 0), stop=(ci == cn - 1))
                    if q == 1:
                        st = stp.tile([P, 2, D_H1], bf16, tag="st2")
                        nc.vector.tensor_copy(st[:], agg2[:])
                        nc.sync.dma_start(
                            p2[g - 1:g + 1].rearrange("g p f -> p g f"), st[:])

            if "noag" not in abl:
                nc.gpsimd.collective_compute(
                    "ReduceScatter", mybir.AluOpType.add,
                    replica_groups=[list(range(NCORES))],
                    ins=[p2.opt()], outs=[m2.opt()])

            # ================= Layer 2 own blocks + t3 =================
            for j in range(NB if "l2o" not in abl else 0):
                mt = smp.tile([P, D_H1], bf16, tag="mt")
                nc.sync.dma_start(mt[:], m2[j])
                mts = smp.tile([P, D_H1], bf16, tag="mts")
                nc.vector.tensor_scalar(mts[:], mt[:], invd_sb[:, j:j + 1],
                                        None, MUL)
                meanT = smp.tile([P, 2, P], bf16, tag="meanT2")
                for k in range(2):
                    tpp = psT.tile([P, P], f32, tag="tp")
                    nc.tensor.matmul(tpp[:], mts[:, k * P:(k + 1) * P],
                                     ident[:], start=True, stop=True)
                    nc.vector.tensor_copy(meanT[:, k, :], tpp[:])

                jsl = slice(j * P, (j + 1) * P)
                hT = htp.tile([P, 8, P], bf16, tag="hT")
                for s in range(8):
                    hp = psT.tile([P, P], f32, tag="tp")
                    sl = slice(s * P, (s + 1) * P)
                    nc.tensor.matmul(hp[:], w2l_sb[:, 0, sl], meanT[:, 0, :],
                                     start=True, stop=False)
                    nc.tensor.matmul(hp[:], w2l_sb[:, 1, sl], meanT[:, 1, :],
                                     start=False, stop=False)
                    nc.tensor.matmul(hp[:], w2r_sb[:, 0, sl],
                                     h1T_res[:, 0, jsl], start=False, stop=False)
                    nc.tensor.matmul(hp[:], w2r_sb[:, 1, sl],
                                     h1T_res[:, 1, jsl], start=False, stop=True)
                    nc.vector.tensor_scalar(hT[:, s, :], hp[:],
                                            b2t_sb[:, s:s + 1], 0.0, ADD, MAX)

                tr = psO.tile([P, D_H1], f32, tag="outp")
                for s in range(8):
                    nc.tensor.matmul(tr[:, 0:P], hT[:, s, :], w3lr_sb[:, s, :],
                                     start=(s == 0), stop=False)
                nc.tensor.matmul(tr[:, 0:P], ones_t[:], b3_sb[:],
                                 start=False, stop=True)
                t3blk = smp.tile([P, P], bf16, tag="t3blk")
                nc.vector.tensor_copy(t3blk[:, 0:D_OUT], tr[:, 0:D_OUT])
                nc.vector.memset(t3blk[:, D_OUT:P], 0.0)
                nc.vector.tensor_copy(r3_res[:, j, :], tr[:, D_OUT:P])
                nc.sync.dma_start(t3_own[j * P:(j + 1) * P, :], t3blk[:])

            # ========== Layer 3 partial sums for ALL dst blocks ==========
            if "l3p" not in abl:
                state["next"] = 0
                state["tiles"] = {}
                agg3 = None
                for g in range(NBLK):
                    c0, cn = cstart[g], int(C2[g])
                    need_chunks(gp3, t3_own[:], P, "gath3",
                                min(c0 + cn + AHEAD, NCH2), False)
                    q = g % 4
                    if q == 0:
                        agg3 = psO.tile([P, D_H1], f32, tag="outp")
                    for ci in range(cn):
                        c = c0 + ci
                        gt = state["tiles"][int(chunk_grp[c])]
                        nc.tensor.matmul(
                            agg3[:, q * D_OUT:(q + 1) * D_OUT], ohc2[:, c, :],
                            gt[:, int(chunk_off[c]), 0:D_OUT],
                            start=(ci == 0), stop=(ci == cn - 1))
                    if q == 3:
                        st = stp.tile([P, 4 * D_OUT], bf16, tag="st3")
                        nc.vector.tensor_copy(st[:], agg3[:])
                        nc.sync.dma_start(
                            p3[g - 3:g + 1].rearrange("g p f -> p g f"), st[:])

            if "noag" not in abl:
                nc.gpsimd.collective_compute(
                    "ReduceScatter", mybir.AluOpType.add,
                    replica_groups=[list(range(NCORES))],
                    ins=[p3.opt()], outs=[m3.opt()])

            # ============ Layer 3 own rows + log_softmax ============
            nc.vector.memset(ssum_res[:], 1.0)
            for j in range(NB if "l3o" not in abl else 0):
                o3 = smp.tile([P, D_OUT], bf16, tag="o3")
                nc.sync.dma_start(o3[:], m3[j])
                y = smp.tile([P, D_OUT], f32, tag="y")
                nc.vector.scalar_tensor_tensor(
                    y[:], o3[:], invd_sb[:, j:j + 1], r3_res[:, j, :],
                    MUL, ADD)
                negm = smp.tile([P, 1], f32, tag="negm")
                nc.vector.tensor_reduce(negm[:], y[:], AXX, MAX, negate=True)
                e = smp.tile([P, D_OUT], f32, tag="e")
                nc.scalar.activation(e[:], y[:], EXP, bias=negm[:, 0:1],
                                     scale=1.0,
                                     accum_out=ssum_res[:, j:j + 1])
                nc.vector.tensor_scalar(ob_res[:, j, :], y[:], negm[:, 0:1],
                                        None, ADD)
            nc.scalar.activation(ls_res[:], ssum_res[:], LN)
            for j in range(NB if "l3o" not in abl else 0):
                ob = smp.tile([P, D_OUT], f32, tag="ob2")
                nc.vector.tensor_scalar(ob[:], ob_res[:, j, :],
                                        ls_res[:, j:j + 1], None, SUB)
                nc.sync.dma_start(outp[j * P:(j + 1) * P, :], ob[:])

    nc.compile()
    return nc


def _wrap16(a):
    """idx i -> partition i%16, col i//16; replicated to 128 partitions."""
    w = a.reshape(-1, 16).T
    return np.ascontiguousarray(np.tile(w, (8, 1)))


def _balanced_perm(deg):
    """Assign nodes to 80 blocks of 128 so block in-degree sums are even."""
    import heapq
    nblk = NPAD // P
    order = np.argsort(-deg, kind="stable")
    heap = [(0, 0, g) for g in range(nblk)]
    heapq.heapify(heap)
    newpos = np.empty(NPAD, np.int64)
    fill = np.zeros(nblk, np.int64)
    for n in order:
        s, _, g = heapq.heappop(heap)
        newpos[n] = g * P + fill[g]
        fill[g] += 1
        if fill[g] < P:
            heapq.heappush(heap, (s + int(deg[n]), int(fill[g]), g))
    return newpos


def _prep(x, edge_index):
    src = np.asarray(edge_index[0], dtype=np.int64)
    dst = np.asarray(edge_index[1], dtype=np.int64)
    deg = np.bincount(dst, minlength=NPAD).astype(np.float64)
    invdeg_n = (1.0 / np.maximum(deg, 1.0)).astype(np.float32)

    newpos = _balanced_perm(deg)
    oldnode = np.empty(NPAD, np.int64)
    oldnode[newpos] = np.arange(NPAD)
    psrc = newpos[src]
    pdst = newpos[dst]

    # ---------- L1: in-edges grouped by own dst block ----------
    order = np.argsort(pdst, kind="stable")
    dsts = pdst[order]
    srcs = psrc[order]
    starts = np.searchsorted(dsts, np.arange(0, NPAD + P, P))
    cnt = starts[1:] - starts[:-1]
    MC1 = max(1, int(np.ceil(cnt.max() / P)))

    xp = np.zeros((NPAD, D_IN), dtype=np.float32)
    xp[:N_NODES] = x
    xp = xp[oldnode]           # permuted node order

    per_core_l1 = []
    for r in range(NCORES):
        gparts, dparts = [], []
        for j in range(NB):
            g = r * NB + j
            lo, hi = starts[g], starts[g + 1]
            n = hi - lo
            o2 = lo + np.argsort(srcs[lo:hi], kind="stable")
            sg = np.zeros(MC1 * P, dtype=np.int16)
            dg = np.full(MC1 * P, -1.0, dtype=np.float32)
            sg[:n] = srcs[o2].astype(np.int16)
            dg[:n] = (dsts[o2] - g * P).astype(np.float32)
            gparts.append(_wrap16(sg))
            dparts.append(np.ascontiguousarray(dg.reshape(MC1, P).T))
        per_core_l1.append((
            np.concatenate(gparts, axis=1),
            np.concatenate(dparts, axis=1),
        ))

    # ---------- L2/L3 partial: out-edges grouped by (src core, dst blk) ----
    srcc = psrc // PER_CORE
    cellcnt = np.bincount(srcc * NBLK + pdst // P,
                          minlength=NCORES * NBLK).reshape(NCORES, NBLK)
    C2 = np.maximum(1, np.ceil(cellcnt.max(axis=0) / P)).astype(np.int64)
    cstart = np.concatenate([[0], np.cumsum(C2)])
    NCH2 = int(cstart[-1])

    per_core_l2 = []
    for r in range(NCORES):
        mask = srcc == r
        es, ed = psrc[mask], pdst[mask]
        o = np.argsort(ed, kind="stable")
        es, ed = es[o], ed[o]
        bstarts = np.searchsorted(ed, np.arange(0, NPAD + P, P))
        sg = np.zeros(NCH2 * P, np.int16)
        dg = np.full(NCH2 * P, -1.0, np.float32)
        for g in range(NBLK):
            lo, hi = bstarts[g], bstarts[g + 1]
            n = hi - lo
            o2 = lo + np.argsort(es[lo:hi], kind="stable")
            s0 = int(cstart[g]) * P
            sg[s0:s0 + n] = (es[o2] - r * PER_CORE).astype(np.int16)
            dg[s0:s0 + n] = (ed[o2] - g * P).astype(np.float32)
        per_core_l2.append((
            _wrap16(sg),
            np.ascontiguousarray(dg.reshape(NCH2, P).T),
        ))

    # per-core inverse degree of own nodes: [P, NB] (partition p, block j)
    invd_own = invdeg_n[oldnode].reshape(NCORES, NB, P)
    invd_own = [np.ascontiguousarray(invd_own[r].T) for r in range(NCORES)]

    return xp, per_core_l1, MC1, per_core_l2, tuple(int(c) for c in C2), \
        invd_own, newpos


def _make_in_maps(x, edge_index, w1l, w1r, b1, w2l, w2r, b2, w3l, w3r, b3):
    x = np.ascontiguousarray(np.asarray(x, dtype=np.float32))
    xp, pc1, MC1, pc2, C2, invd_own, newpos = _prep(x, np.asarray(edge_index))

    iota = np.tile(np.arange(P, dtype=np.float32), (P, 1))
    b1v = np.asarray(b1, np.float32).reshape(-1)
    b2v = np.asarray(b2, np.float32).reshape(-1)
    xbf = xp.astype(BF)
    common = {
        "xbf": xbf,
        "w1l": np.asarray(w1l, np.float32).astype(BF),
        "w1r": np.asarray(w1r, np.float32).astype(BF),
        "b1": b1v.reshape(1, D_H1).astype(BF),
        "b1t": np.ascontiguousarray(b1v.reshape(2, P).T),
        "w2l": np.asarray(w2l, np.float32).astype(BF),
        "w2r": np.asarray(w2r, np.float32).astype(BF),
        "b2t": np.ascontiguousarray(b2v.reshape(8, P).T),
        "w3lr": np.ascontiguousarray(np.concatenate(
            [np.asarray(w3l, np.float32), np.asarray(w3r, np.float32)],
            axis=1)).astype(BF),
        "b3pad": np.concatenate(
            [np.zeros(D_OUT, np.float32),
             np.asarray(b3, np.float32).reshape(-1)]).reshape(1, P).astype(BF),
        "iota_in": iota,
        "partidx": np.arange(P, dtype=np.float32).reshape(P, 1),
    }
    in_maps = []
    for r in range(NCORES):
        g1, d1 = pc1[r]
        g2, d2 = pc2[r]
        m = dict(common)
        m["xownT"] = np.ascontiguousarray(
            xbf[r * PER_CORE:(r + 1) * PER_CORE].T)
        m["gidx1"] = g1
        m["dstloc1"] = d1
        m["gidx2"] = g2
        m["dstloc2"] = d2
        m["invd_own"] = invd_own[r]
        in_maps.append(m)
    return in_maps, (MC1, C2), newpos


def kernel(x, edge_index, w1l, w1r, b1, w2l, w2r, b2, w3l, w3r, b3):
    global LAST_RESULTS
    import os
    from concourse.bass_utils import run_bass_kernel_spmd

    if os.environ.get("BASS_TRACE"):
        try:
            import antenv.axon_hooks  # noqa: F401
        except ImportError:
            os.environ.pop("BASS_TRACE", None)  # no NTFF hook here

    in_maps, key, newpos = _make_in_maps(x, edge_index, w1l, w1r, b1, w2l,
                                         w2r, b2, w3l, w3r, b3)
    if key not in _CACHE:
        _CACHE[key] = _build(*key)
    nc = _CACHE[key]

    res = run_bass_kernel_spmd(nc, in_maps, core_ids=list(range(NCORES)))
    LAST_RESULTS = res
    out = np.concatenate([res.results[r]["out"] for r in range(NCORES)], axis=0)
    return np.ascontiguousarray(out[newpos[:N_NODES]])
